# revision 55
# baseline (speedup 1.0000x reference)
"""Bass/Tile TRN2 kernel for BantamAttention (sliding-window GQA attention).

Sharding: 8 cores, tensor-parallel on heads. Core c gets q heads 4c..4c+3,
kv head c (Wq/Wk/Wv column slices, per-kv-head cache slice, Wo row slice).
Each core computes a partial (1024, 4096) output (its heads' contribution
through Wo); the host sums the 8 partials (the Wo-row-parallel unshard).

v2 design (vs v1 baseline at 372us -> ~257us):
- fp16 everywhere instead of bf16 (same PE rate, 8x finer mantissa)
- softmax denominator off the PE hot path: es blocks accumulate on DVE
  (heads 0-1) or via tiny inline ones-matmuls placed in ACT-throttled
  slots (heads 2-3); one small ones-matmul + reciprocal +
  gpsimd partition_broadcast per head replaces 54us of PE ones-matmuls
- causal skip on the 8 new-key blocks (suffix columns only) with a single
  128x128 triangular mask for the diagonal tile
- software-pipelined emission: chunk-major (k,q0,q1) projection pass with
  the hT DMA hidden under it; a v-pass + head-0 scores/AV window; q2/q3
  projection passes woven per-block into the attention windows (each pass
  head-starts in the previous window's ACT-throttled tail slots). hT
  persists in SBUF so later passes do no DMA. The A1 psum accumulators
  live inside the attention pools' tag rotations so no pool-close barrier
  separates phases, and dummy warmup matmuls ramp the PE p-state during
  the first DMAs. The PE queue stays dense; ACT exp pacing bounds the
  few remaining idle slots.

Schedule (windows of 32 slots; AV trails scores by LAG blocks):
  A1 : chunk-major k+q0+q1 projection (6 psum banks, DMA-paced)
  W-1: v-pass + scores h0 + AV(h0) + v transposes + q2 head-start
  w0 : scores h1 + q2 body + AV(h0 tail/norm h0) + AV(h1) + q3 head-start
  w1 : scores h2 + q3 body + ... h1 ... AV(h2) + inline pd(h2)
  w2 : scores h3 + AV(h2 tail/norm h2) + AV(h3) + inline pd(h3)
  w3 : AV(h3 tail) + recip/norm h3 + Wo prefetch
  WO : output projection; first four tiles prefilled with h0..h2 inside
       the attention pool scope while the norm3 chain lands; psum->fp16
       copies alternate DVE/ACT; host sums the 8 fp16 partials in fp32
"""

import numpy as np
from contextlib import ExitStack

import concourse.bass as bass
from concourse import bacc
import concourse.mybir as mybir
import concourse.tile as tile
from concourse.bass_utils import run_bass_kernel_spmd
from concourse.masks import make_identity

F16 = mybir.dt.float16
F32 = mybir.dt.float32
EXP = mybir.ActivationFunctionType.Exp

Q = 1024          # new tokens
DM = 4096         # model dim
D = 128           # head dim
HPC = 4           # q heads per core
P = 4096          # past length
NCORES = 8
SINKS = 4
NKEEP = 4096      # kept keys (sliding window)
NPAST = 3072      # kept keys from the cache (sinks + tail)
NJB = NKEEP // D          # 32 key blocks
NJB_PAST = NPAST // D     # 24 from cache, 8 from new tokens
NCHUNK = DM // D          # 32 contraction chunks for projections
SCALE = float(1.0 / np.sqrt(D))
PAST_TAIL0 = P - (NPAST - SINKS)   # 1028: first kept cache row after sinks
LAG = 10          # AV trails scores by LAG blocks (es decoupling depth)

TRACE = False
LAST_RESULT = None

# wqkv host column layout: [k | q0 | q1 | v | q2 | q3] (128 cols each)
COL_V = 384
COL_QH = {2: 512, 3: 640}


def _build():
    nc = bacc.Bacc()
    hT = nc.declare_dram_parameter("hT", [DM, Q], F16, isOutput=False)
    wqkv = nc.declare_dram_parameter("wqkv", [DM, (HPC + 2) * D], F16, isOutput=False)
    wo = nc.declare_dram_parameter("wo", [D, HPC, DM], F16, isOutput=False)
    pkT = nc.declare_dram_parameter("pkT", [D, P], F16, isOutput=False)
    pv = nc.declare_dram_parameter("pv", [P, D], F16, isOutput=False)
    cosT = nc.declare_dram_parameter("cosT", [D, Q], F16, isOutput=False)
    sinE = nc.declare_dram_parameter("sinE", [D, Q], F16, isOutput=False)
    tri = nc.declare_dram_parameter("tri", [D, D], F16, isOutput=False)
    onesd = nc.declare_dram_parameter("ones", [D, 1], F16, isOutput=False)
    outp = nc.declare_dram_parameter("out", [Q, DM], F16, isOutput=True)

    wqkv_r = wqkv.rearrange("(c p) n -> p c n", p=D)

    with ExitStack() as ctx:
        tc = ctx.enter_context(tile.TileContext(nc))
        const = ctx.enter_context(tc.tile_pool(name="const", bufs=1))
        persist = ctx.enter_context(tc.tile_pool(name="persist", bufs=1))
        hpool = ctx.enter_context(tc.tile_pool(name="hpool", bufs=1))
        dr = ctx.enter_context(tc.tile_pool(name="dr", bufs=2))
        win = ctx.enter_context(tc.tile_pool(name="win", bufs=1))
        esp = ctx.enter_context(tc.tile_pool(name="esp", bufs=12))
        wsb = ctx.enter_context(tc.tile_pool(name="wsb", bufs=3))
        osb = ctx.enter_context(tc.tile_pool(name="osb", bufs=12))

        wsrc = const.tile([D, D], F16, tag="wsrc")
        nc.vector.memset(wsrc[:, :], 1.0)
        ident = const.tile([D, D], F16, tag="ident")
        make_identity(nc, ident[:, :])
        ones_sb = const.tile([D, 1], F16, tag="ones_sb")
        tri_sb = const.tile([D, D], F16, tag="tri_sb")
        cos_t = const.tile([D, Q], F16, tag="cos")
        sin_t = const.tile([D, Q], F16, tag="sin")

        qT = [persist.tile([D, Q], F16, tag=f"qT{h}", name=f"qT{h}") for h in range(HPC)]
        kT_new = persist.tile([D, Q], F16, tag="kT_new")
        vT_new = persist.tile([D, Q], F16, tag="vT_new")
        kT_past = persist.tile([D, NPAST], F16, tag="kT_past")
        v_keep = persist.tile([D, NJB * D], F16, tag="v_keep")
        oT = [persist.tile([D, Q], F16, tag=f"oT{h}", name=f"oT{h}") for h in range(HPC)]
        accd = [persist.tile([D, Q], F16, tag=f"accd{i}", name=f"accd{i}")
                for i in range(2)]
        rc = persist.tile([1, Q], F16, tag="rc")
        bc = persist.tile([D, Q], F16, tag="bc")
        # hT chunk groups: two pairs up front (fast start), then quads
        # (amortizes per-DMA overhead; A1 is DMA-bound)
        h_p = [hpool.tile([D, 2, Q], F16, tag=f"hp{i}", name=f"hp{i}")
               for i in range(2)]
        h_q = [hpool.tile([D, 4, Q], F16, tag=f"hq{i}", name=f"hq{i}")
               for i in range(7)]

        def h_c(c):
            if c < 4:
                return h_p[c // 2][:, c % 2, :]
            return h_q[(c - 4) // 4][:, (c - 4) % 4, :]

        wV = win.tile([D, NCHUNK, D], F16, tag="wV")

        def rope_drain(pacc_lo, pacc_hi, dst, eng=None, tg=""):
            eng = eng or nc.vector
            for hi, pacc in ((0, pacc_lo), (1, pacc_hi)):
                s = slice(hi * 512, (hi + 1) * 512)
                ta = dr.tile([D, 512], F32, tag=f"ropeA{tg}", name="ropeA")
                tb = dr.tile([D, 512], F32, tag=f"ropeB{tg}", name="ropeB")
                eng.tensor_mul(ta[:, :], pacc[:, :], cos_t[:, s])
                eng.tensor_mul(tb[0:64, :], pacc[64:128, :], sin_t[0:64, s])
                eng.tensor_mul(tb[64:128, :], pacc[0:64, :], sin_t[64:128, s])
                eng.tensor_add(dst[:, s], ta[:, :], tb[:, :])

        def rope_drain_pool(pacc_lo, pacc_hi, dst):
            # GPSIMD cannot read PSUM on real hardware; keep on DVE
            rope_drain(pacc_lo, pacc_hi, dst)

        # ================= attention pools (opened before A1 so the A1
        # accumulators live in the same tag rotations — slot-reuse deps are
        # then per-bank instead of a pool-close barrier) ====================
        att_stack = ExitStack()
        psp = att_stack.enter_context(tc.tile_pool(name="att_ps", bufs=2, space="PSUM"))
        pop = att_stack.enter_context(tc.tile_pool(name="po_ps", bufs=1, space="PSUM"))
        auxp = att_stack.enter_context(tc.tile_pool(name="aux_ps", bufs=2, space="PSUM"))
        wqpool = att_stack.enter_context(tc.tile_pool(name="wqpool", bufs=2))

        # ================= A1: k + q0 + q1 projection (chunk-major) ========
        with tc.tile_pool(name="a1in", bufs=4) as a1in:
            # q0+k accumulators take the two "ps" slots (banks 0-3), q1 the
            # "po" slot (banks 4-5); banks 6/7 stay free for the v-pass
            acc_q0t = psp.tile([D, Q], F32, tag="ps", name="acc_q0t")
            acc_kt = psp.tile([D, Q], F32, tag="ps", name="acc_kt")
            acc_q1t = pop.tile([D, Q], F32, tag="po", name="acc_q1t")
            acc_q0 = [acc_q0t[:, 0:512], acc_q0t[:, 512:Q]]
            acc_k = [acc_kt[:, 0:512], acc_kt[:, 512:Q]]
            acc_q1 = [acc_q1t[:, 0:512], acc_q1t[:, 512:Q]]
            hT_r = hT.rearrange("(i s p) q -> p (i s) q", p=D, s=2)
            hT_r4 = hT.rearrange("(i s p) q -> p (i s) q", p=D, s=4)
            wA_r = wqkv.rearrange("(i s p) n -> p (i s) n", p=D, s=2)
            # warm the PE clock during the first DMAs: dummy matmuls on the
            # (locally initialized) identity tile ramp the p-state so real
            # work starts at full speed
            # warmup matmuls: content is irrelevant (the accumulator is
            # reset by the first real start=True matmul), they only ramp the
            # PE clock during the initial DMA wait
            for w in range(16):
                nc.tensor.matmul(acc_q1[0][:, 0:128], wsrc[:, :], wsrc[:, :],
                                 start=True, stop=True)
            wA_r4 = wqkv.rearrange("(i s p) n -> p (i s) n", p=D, s=4)
            wts = []
            for i in range(NCHUNK // 2):
                if i == 0:
                    wt = a1in.tile([D, 2, 384], F16, tag="wA2", name="wA2", bufs=2)
                    # single-chunk DMAs up front so the PE starts sooner
                    nc.sync.dma_start(wt[:, 0, :], wA_r[:, 0, 0:384])
                    nc.sync.dma_start(h_p[0][:, 0, :], hT_r[:, 0, :])
                    nc.sync.dma_start(wt[:, 1, :], wA_r[:, 1, 0:384])
                    nc.sync.dma_start(h_p[0][:, 1, :], hT_r[:, 1, :])
                    wts.append((wt, 0))
                elif i == 1:
                    wt = a1in.tile([D, 2, 384], F16, tag="wA2", name="wA2", bufs=2)
                    nc.sync.dma_start(h_p[1][:, :, :], hT_r[:, 2:4, :])
                    nc.sync.dma_start(wt[:, :, :], wA_r[:, 2:4, 0:384])
                    wts.append((wt, 0))
                else:
                    if i % 2 == 0:
                        qi = (i - 2) // 2
                        if qi == 0:
                            nc.sync.dma_start(h_q[0][:, 0:2, :],
                                              hT_r4[:, 4:6, :])
                            nc.sync.dma_start(h_q[0][:, 2:4, :],
                                              hT_r4[:, 6:8, :])
                        else:
                            nc.sync.dma_start(h_q[qi][:, :, :],
                                              hT_r4[:, 4 * qi + 4:4 * qi + 8, :])
                        wt = a1in.tile([D, 4, 384], F16, tag="wA4", name="wA4", bufs=2)
                        nc.sync.dma_start(wt[:, :, :],
                                          wA_r4[:, 2 * i:2 * i + 4, 0:384])
                        wts.append((wt, 0))
                    else:
                        wts.append((wts[-1][0], 2))
                if i == 11:
                    nc.sync.dma_start(cos_t[:, :], cosT[:, :])
                    nc.sync.dma_start(sin_t[:, :], sinE[:, :])
                    nc.sync.dma_start(ones_sb[:, :], onesd[:, :])
                    nc.sync.dma_start(tri_sb[:, :], tri[:, :])
                if i == 12:
                    nc.sync.dma_start(wV[:, 0:16, :],
                                      wqkv_r[:, 0:16, COL_V:COL_V + D])
                wtt, sbase = wts[i]
                for s2 in range(2):
                    c = 2 * i + s2
                    st, sp = c == 0, c == NCHUNK - 1
                    for half in range(2):
                        s = slice(half * 512, (half + 1) * 512)
                        nc.tensor.matmul(acc_q0[half][:, :],
                                         wtt[:, sbase + s2, 128:256],
                                         h_c(c)[:, s], start=st, stop=sp)
                        nc.tensor.matmul(acc_k[half][:, :],
                                         wtt[:, sbase + s2, 0:128],
                                         h_c(c)[:, s], start=st, stop=sp)
                        nc.tensor.matmul(acc_q1[half][:, :],
                                         wtt[:, sbase + s2, 256:384],
                                         h_c(c)[:, s], start=st, stop=sp)
            # kT_past heads the post-A1 queue: scores(0,0) needs its first
            # block no later than the q0 rope drain completes
            nc.sync.dma_start(kT_past[:, 0:SINKS], pkT[:, 0:SINKS])
            nc.sync.dma_start(kT_past[:, SINKS:SINKS + 1020],
                              pkT[:, PAST_TAIL0:PAST_TAIL0 + 1020])
            nc.sync.dma_start(wV[:, 16:NCHUNK, :],
                              wqkv_r[:, 16:NCHUNK, COL_V:COL_V + D])
            nc.sync.dma_start(kT_past[:, SINKS + 1020:NPAST],
                              pkT[:, PAST_TAIL0 + 1020:P])
            rope_drain(acc_q0[0], acc_q0[1], qT[0])
            rope_drain_pool(acc_k[0], acc_k[1], kT_new)
            rope_drain(acc_q1[0], acc_q1[1], qT[1])

        # ================= attention + interleaved q/v passes ==============
        if True:   # attention emission (pools opened above, before A1)

            es_tiles = {}

            def emit_scores(h, jb):
                ps = psp.tile([D, Q], F32, tag="ps", name="ps")
                if jb < NJB_PAST:
                    ksl = kT_past[:, jb * D:(jb + 1) * D]
                    c0 = 0
                else:
                    t = jb - NJB_PAST
                    ksl = kT_new[:, t * D:(t + 1) * D]
                    c0 = t * D
                if c0 < 512:
                    nc.tensor.matmul(ps[:, c0:512], ksl, qT[h][:, c0:512],
                                     start=True, stop=True)
                    nc.tensor.matmul(ps[:, 512:Q], ksl, qT[h][:, 512:Q],
                                     start=True, stop=True)
                else:
                    nc.tensor.matmul(ps[:, c0:Q], ksl, qT[h][:, c0:Q],
                                     start=True, stop=True)
                es = esp.tile([D, Q], F16, tag="es", name="es")
                nc.scalar.activation(es[:, c0:Q], ps[:, c0:Q], EXP, scale=SCALE)
                if jb >= NJB_PAST:
                    t = jb - NJB_PAST
                    nc.vector.tensor_mul(es[:, c0:c0 + D], es[:, c0:c0 + D],
                                         tri_sb[:, :])
                    if h < HPC - 2:
                        nc.vector.tensor_add(accd[h % 2][:, c0:Q],
                                             accd[h % 2][:, c0:Q], es[:, c0:Q])
                else:
                    if jb == 0:
                        nc.vector.tensor_copy(accd[h % 2][:, :], es[:, :])
                    else:
                        nc.vector.tensor_add(accd[h % 2][:, :],
                                             accd[h % 2][:, :], es[:, :])
                es_tiles[(h, jb)] = es

            pdt = {}

            def emit_pd_inline(h, t):
                # denominator for the 8 new-key blocks: tiny ones-matmuls
                # accumulated in psum; the accd (past-block) contribution
                # joins at t == 7 with the stop flag
                c0 = t * D
                es = es_tiles[(h, NJB_PAST + t)]
                if t == 0:
                    pdt[h] = [auxp.tile([1, 512], F32, tag="aux",
                                        name=f"pd{h}_{i}") for i in range(2)]
                pd = pdt[h]
                if c0 < 512:
                    nc.tensor.matmul(pd[0][:, c0:512], ones_sb[:, :],
                                     es[:, c0:512], start=(t == 0), stop=False)
                    nc.tensor.matmul(pd[1][:, :], ones_sb[:, :],
                                     es[:, 512:Q], start=(t == 0), stop=False)
                else:
                    nc.tensor.matmul(pd[1][:, c0 - 512:512], ones_sb[:, :],
                                     es[:, c0:Q], start=False, stop=False)
                if t == 7:
                    for half in range(2):
                        s = slice(half * 512, (half + 1) * 512)
                        nc.tensor.matmul(pd[half][:, :], ones_sb[:, :],
                                         accd[h % 2][:, s], start=False,
                                         stop=True)

            def emit_recip(h):
                if h < HPC - 2:
                    pd = psp.tile([1, Q], F32, tag="ps", name=f"pdm{h}")
                    for half in range(2):
                        s = slice(half * 512, (half + 1) * 512)
                        nc.tensor.matmul(pd[:, s], ones_sb[:, :],
                                         accd[h % 2][:, s], start=True,
                                         stop=True)
                    with nc.allow_low_precision(reason="denominator recip"):
                        nc.vector.reciprocal(rc[:, :], pd[0:1, :])
                else:
                    pdp_ = pdt.pop(h)
                    with nc.allow_low_precision(reason="denominator recip"):
                        nc.vector.reciprocal(rc[0:1, 0:512], pdp_[0][0:1, :])
                        nc.vector.reciprocal(rc[0:1, 512:Q], pdp_[1][0:1, :])
                nc.gpsimd.partition_broadcast(bc[:, :], rc[0:1, :])

            def emit_av(h, jb, po):
                es = es_tiles.pop((h, jb))
                vsl = v_keep[:, jb * D:(jb + 1) * D]
                if jb < NJB_PAST:
                    st = jb == 0
                    nc.tensor.matmul(po[:, 0:512], vsl, es[:, 0:512],
                                     start=st, stop=False)
                    nc.tensor.matmul(po[:, 512:Q], vsl, es[:, 512:Q],
                                     start=st, stop=False)
                else:
                    t = jb - NJB_PAST
                    for k in range(t, 8):
                        cs = slice(k * D, (k + 1) * D)
                        nc.tensor.matmul(po[:, cs], vsl, es[:, cs],
                                         start=False, stop=(k == t))

            def emit_norm(h, po):
                for half in range(2):
                    s = slice(half * 512, (half + 1) * 512)
                    nc.vector.tensor_mul(oT[h][:, s], po[:, s], bc[:, s])

            def emit_vtrans(t):
                ptr = psp.tile([D, D], F16, tag="ps", name="ptr")
                nc.tensor.transpose(ptr[:, :], vT_new[:, t * D:(t + 1) * D],
                                    ident[:, :])
                nc.vector.tensor_copy(
                    v_keep[:, (NJB_PAST + t) * D:(NJB_PAST + t + 1) * D],
                    ptr[:, :])

            # ---------------- W-1: v-pass + scores h0 + AV h0 --------------
            nc.sync.dma_start(v_keep[0:SINKS, 0:D], pv[0:SINKS, :])
            nc.sync.dma_start(v_keep[SINKS:D, 0:D],
                              pv[PAST_TAIL0:PAST_TAIL0 + D - SINKS, :])
            for jb in range(1, NJB_PAST):
                r0 = PAST_TAIL0 + jb * D - SINKS
                nc.sync.dma_start(v_keep[:, jb * D:(jb + 1) * D], pv[r0:r0 + D, :])
            wq2 = wqpool.tile([D, NCHUNK, D], F16, tag="wq", name="wq2")
            nc.sync.dma_start(wq2[:, :, :],
                              wqkv_r[:, :, COL_QH[2]:COL_QH[2] + D])
            wq3 = wqpool.tile([D, NCHUNK, D], F16, tag="wq", name="wq3")
            nc.sync.dma_start(wq3[:, :, :],
                              wqkv_r[:, :, COL_QH[3]:COL_QH[3] + D])
            wqs = {2: wq2, 3: wq3}
            qas = {}

            def emit_qchunk(hq, cc):
                if cc == 0:
                    qas[hq] = [auxp.tile([D, 512], F32, tag="aux",
                                         name=f"qa{hq}{i}") for i in range(2)]
                qa = qas[hq]
                st, sp = cc == 0, cc == NCHUNK - 1
                for half in range(2):
                    s = slice(half * 512, (half + 1) * 512)
                    nc.tensor.matmul(qa[half][:, :], wqs[hq][:, cc, :],
                                     h_c(cc)[:, s], start=st, stop=sp)

            acc_v = [auxp.tile([D, 512], F32, tag="aux", name=f"accv{i}")
                     for i in range(2)]

            def v_chunk(c):
                st, sp = c == 0, c == NCHUNK - 1
                for half in range(2):
                    s = slice(half * 512, (half + 1) * 512)
                    nc.tensor.matmul(acc_v[half][:, :], wV[:, c, :],
                                     h_c(c)[:, s], start=st, stop=sp)

            po_cur = pop.tile([D, Q], F32, tag="po", name="po0")
            # v-chunk count per slot sized to each slot's PE deficit vs the
            # ACT exp pace (slots 0..8 have no AV yet)
            V_SCHED = [2, 1, 2, 1, 2, 1, 2, 1, 2, 2, 2, 2, 2] + [1] * 10
            v_next = 0
            for j in range(NJB):
                for _ in range(V_SCHED[j] if j < len(V_SCHED) else 0):
                    v_chunk(v_next)
                    v_next += 1
                if j == 23:
                    nc.vector.tensor_copy(vT_new[:, 0:512], acc_v[0][:, :])
                    nc.vector.tensor_copy(vT_new[:, 512:Q], acc_v[1][:, :])
                if 24 <= j < 28:
                    emit_vtrans(2 * (j - 24))
                    emit_vtrans(2 * (j - 24) + 1)
                emit_scores(0, j)
                if j >= 24:
                    emit_qchunk(2, j - 24)
                if j >= LAG:
                    emit_av(0, j - LAG, po_cur)

            # ---------------- steady windows h=0..3 -------------------------
            for h in range(HPC):
                hn = h + 1
                if h == HPC - 2:
                    wo_pre = [wsb.tile([D, HPC, 512], F16, tag="wo", name=f"wo{nb}")
                              for nb in range(2)]
                    for nb in range(2):
                        nc.sync.dma_start(wo_pre[nb][:, :, :],
                                          wo[:, :, nb * 512:(nb + 1) * 512])
                po_nxt = None
                if hn < HPC:
                    po_nxt = pop.tile([D, Q], F32, tag="po", name=f"po{hn}")
                for j in range(NJB):
                    if j == (1 if h < HPC - 1 else 0):
                        # recip placed early; frees pd slots before reuse
                        emit_recip(h)
                    if hn < HPC:
                        emit_scores(hn, j)
                        if hn >= HPC - 2 and j >= 24:
                            emit_pd_inline(hn, j - 24)
                        if hn + 1 < HPC and j < 24:
                            nq = 8 if hn + 1 == 2 else 4
                            for cc in range(nq + (j * (32 - nq)) // 24,
                                            nq + ((j + 1) * (32 - nq)) // 24):
                                emit_qchunk(hn + 1, cc)
                        if hn + 1 < HPC and j == 24:
                            qa = qas.pop(hn + 1)
                            rope_drain(qa[0], qa[1], qT[hn + 1])
                        if hn + 2 < HPC and j >= 28:
                            emit_qchunk(hn + 2, j - 28)
                    if j < LAG:
                        emit_av(h, NJB - LAG + j, po_cur)
                    if j == LAG:
                        emit_norm(h, po_cur)
                    if hn < HPC and j >= LAG:
                        emit_av(hn, j - LAG, po_nxt)
                    if hn == HPC and j == LAG:
                        break
                po_cur = po_nxt

            # ---- Wo phase, entirely inside the attention scope: pw psum
            # tiles ride the ps/aux/po tag rotations, so the first tiles
            # start as soon as their banks free (no pool-close barrier).
            # The first four (nb=0) tiles prefill h0..h2 while the norm3
            # chain completes.
            po_share = [None]

            def pw_alloc(idx, name):
                kind = idx % 6
                if kind in (0, 3):
                    return psp.tile([D, 512], F32, tag="ps", name=name)
                if kind in (1, 4):
                    return auxp.tile([D, 512], F32, tag="aux", name=name)
                if kind == 2:
                    po_share[0] = pop.tile([D, Q], F32, tag="po", name=name)
                    return po_share[0][:, 0:512]
                return po_share[0][:, 512:Q]

            pw_pre = []
            for ib in range(4):
                pw = pw_alloc([0, 1, 0, 1][ib], f"pwp{ib}")
                for h in range(HPC - 1):
                    nc.tensor.matmul(pw[:, :], oT[h][:, ib * D:(ib + 1) * D],
                                     wo_pre[0][:, h, :], start=(h == 0),
                                     stop=False)
                pw_pre.append(pw)
            for ib, pw in enumerate(pw_pre):
                nc.tensor.matmul(pw[:, :], oT[HPC - 1][:, ib * D:(ib + 1) * D],
                                 wo_pre[0][:, HPC - 1, :], start=False,
                                 stop=True)
                ot = osb.tile([D, 512], F16, tag="ot")
                if ib % 2 == 0:
                    nc.vector.tensor_copy(ot[:, :], pw[:, :])
                else:
                    nc.scalar.activation(ot[:, :], pw[:, :],
                                         mybir.ActivationFunctionType.Copy)
                nc.sync.dma_start(outp[ib * D:(ib + 1) * D, 0:512], ot[:, :])

            k = 0
            wot = {0: wo_pre[0], 1: wo_pre[1]}
            for nb in range(8):
                if nb + 2 < 8:      # prefetch two slabs ahead
                    wot[nb + 2] = wsb.tile([D, HPC, 512], F16, tag="wo",
                                           name=f"wo{nb + 2}")
                    nc.sync.dma_start(wot[nb + 2][:, :, :],
                                      wo[:, :, (nb + 2) * 512:(nb + 3) * 512])
                wo_t = wot.pop(nb)
                for ib in range(8):
                    if nb == 0 and ib < 4:
                        continue   # prefilled above
                    pw = pw_alloc(k, f"pw{nb}_{ib}")
                    for h in range(HPC):
                        nc.tensor.matmul(pw[:, :], oT[h][:, ib * D:(ib + 1) * D],
                                         wo_t[:, h, :], start=(h == 0),
                                         stop=(h == HPC - 1))
                    ot = osb.tile([D, 512], F16, tag="ot")
                    if k % 2 == 0:
                        nc.vector.tensor_copy(ot[:, :], pw[:, :])
                    else:
                        nc.scalar.activation(ot[:, :], pw[:, :],
                                             mybir.ActivationFunctionType.Copy)
                    k += 1
                    nc.sync.dma_start(
                        outp[ib * D:(ib + 1) * D, nb * 512:(nb + 1) * 512],
                        ot[:, :])
        att_stack.close()
    nc.compile()
    return nc


_cache = {}


def kernel(**inputs):
    global LAST_RESULT
    hidden = np.asarray(inputs["hidden"], np.float32)
    Wq = np.asarray(inputs["Wq"], np.float32)
    Wk = np.asarray(inputs["Wk"], np.float32)
    Wv = np.asarray(inputs["Wv"], np.float32)
    Wo = np.asarray(inputs["Wo"], np.float32)
    past_k = np.asarray(inputs["past_k"], np.float32)
    past_v = np.asarray(inputs["past_v"], np.float32)
    cos = np.asarray(inputs["cos"], np.float32)
    sin = np.asarray(inputs["sin"], np.float32)

    f16 = np.float16
    hT = np.ascontiguousarray(hidden[0].T).astype(f16)
    cosT = np.ascontiguousarray(cos[P:P + Q].T)
    sinT = np.ascontiguousarray(sin[P:P + Q].T)
    sinT[:64] *= -1.0
    tri = np.triu(np.ones((D, D), dtype=f16))
    ones = np.ones((D, 1), dtype=f16)

    if "nc" not in _cache:
        _cache["nc"] = _build()
    nc = _cache["nc"]

    in_maps = []
    for c in range(NCORES):
        wq_c = Wq[:, c * HPC * D:(c + 1) * HPC * D]
        # column layout [k | q0 | q1 | v | q2 | q3]
        wqkv_c = np.concatenate([
            Wk[:, c * D:(c + 1) * D],
            wq_c[:, 0:D],
            wq_c[:, D:2 * D],
            Wv[:, c * D:(c + 1) * D],
            wq_c[:, 2 * D:3 * D],
            wq_c[:, 3 * D:4 * D],
        ], axis=1)
        in_maps.append({
            "hT": hT,
            "wqkv": np.ascontiguousarray(wqkv_c).astype(f16),
            "wo": np.ascontiguousarray(
                Wo[c * HPC * D:(c + 1) * HPC * D, :].reshape(HPC, D, DM)
                .transpose(1, 0, 2)).astype(f16),
            "pkT": np.ascontiguousarray(past_k[0, c].T).astype(f16),
            "pv": np.ascontiguousarray(past_v[0, c]).astype(f16),
            "cosT": cosT.astype(f16),
            "sinE": sinT.astype(f16),
            "tri": tri,
            "ones": ones,
        })
    res = run_bass_kernel_spmd(nc, in_maps, list(range(NCORES)), trace=TRACE)
    LAST_RESULT = res
    total = np.zeros((Q, DM), np.float32)
    for r in res.results:
        total += np.asarray(r["out"]).astype(np.float32)
    return total.reshape(1, Q, DM)
